# revision 37
# baseline (speedup 1.0000x reference)
"""Trainium2 Bass kernel for PoissonGaussianReadout.

Computation (per reference):
  out[b, n] = elu( sum_c bilinear_sample(x[b, c], mu[n]) * W[n, c] + bias[n] ) + 1

Sharding: data-parallel over batch B=32 across 8 cores (4 images per core).
Every core processes all N=8192 neurons for its 4 images.

Device strategy per core ("rect-B": channel dot on TensorE first, then
bilinear interpolation as a small DVE weighted sum; 73us vs 477us for the
dma_gather baseline):
  - Neurons are host-sorted by (y0//8, x0, y0), packed into 64 tiles of
    128.  A tile's corner pixels then span one small rectangle (~9x12 px,
    FD~84 avg; recursively split only if over a PSUM half-bank).
  - x is host-transposed channel-major: x[c_part, c_chunk, b, pix],
    pixel-contiguous so a rect AP spans rows freely.
  - TensorE: Y[slot, b, rect-px] = sum_c W[slot, c] * x[c, b, rect-px] —
    stationary = per-tile W chunk [128c x 128n], moving = the rect pixels,
    PSUM-accumulated over the 2 c-chunks.  Per-b-pair matmuls keep each
    output inside one PSUM bank.
  - ScalarE drains PSUM f32 -> SBUF bf16 per bin (rects first-fit packed
    into <=256-col PSUM half-bank bins).
  - DVE: z[slot, b] = sum_px Y[slot, b, px] * S'[slot, px] where S' holds
    the 4 bilinear corner weights (zero elsewhere) — tensor_tensor mult
    (2x bf16) + tensor_reduce, per tile pair.
  - Epilogue: out = exp(min(z+bias,0)) + max(z+bias,0)  (== elu(z)+1).
"""

import numpy as np
import ml_dtypes

B, C, H, Wd, N = 32, 256, 64, 64, 8192
NCORES = 8
BL = B // NCORES          # 4 images per core
P = 128                   # partitions / neurons per tile
NT = N // P               # 64 neuron tiles
NPB = 4                   # x DMA chunks
NPIX = H * Wd             # 4096
PBSZ = 1024               # pixels per x DMA chunk

_PROGRAM = None


def _build_program(meta):
    """meta: dict with per-tile rect/bin structure (see _host_prep)."""
    import concourse.bass as bass
    import concourse.mybir as mybir
    import concourse.tile as tile

    bf16 = mybir.dt.bfloat16
    f32 = mybir.dt.float32

    tiles = meta["tiles"]        # per tile: list of bins; bin = list of rects
                                 # rect = (br, rmin, nr, xmin, xl, coloff)
    fdt = meta["fdt"]            # per tile FD (cols)
    fdq = meta["fdq"]            # per pair padded FD
    soff = meta["soff"]          # per pair S' offset (elements per partition)
    ssz = meta["ssz"]
    xoff = meta["xoff"]
    xsz = meta["xsz"]
    fdmax = max(fdq)

    nc = bass.Bass("TRN2")

    xt = nc.dram_tensor("xt", [P, xsz], bf16, kind="ExternalInput")
    ws = nc.dram_tensor("ws", [P, NT * 2 * P], bf16, kind="ExternalInput")
    ss = nc.dram_tensor("ss", [P, ssz], bf16, kind="ExternalInput")
    biasr = nc.dram_tensor("biasr", [P, NT], f32, kind="ExternalInput")
    out = nc.dram_tensor("out", [P, NT * BL], f32, kind="ExternalOutput")

    TG = 8  # tiles per S/W DMA chunk
    XB = [0, 640, 1664, 2688, 3712, 4096]  # x chunk bounds: small first chunk

    with tile.TileContext(nc) as tc:
        with (
            tc.tile_pool(name="const", bufs=1) as cpool,
            tc.tile_pool(name="fpool", bufs=3) as fpool,
            tc.tile_pool(name="upool", bufs=3) as upool,
            tc.tile_pool(name="psum", bufs=2, space="PSUM") as ppool,
        ):
            x_sb = cpool.tile([P, xsz], bf16)
            s_sb = cpool.tile([P, ssz], bf16)
            w_sb = cpool.tile([P, NT, 2, P], bf16)
            bias_sb = cpool.tile([P, NT], f32)
            z_sb = cpool.tile([P, NT, BL], f32)

            # DMA interleave: fine-grained chunks early so tile-0 deps land
            # fast, coarser later.  Chunks are tile ranges.
            chunks = [(0, 2), (2, 4), (4, 8)] + [
                (t, t + TG) for t in range(8, NT, TG)
            ]
            for t0, t1 in chunks:
                s0, s1 = soff[t0 // 2], soff[t1 // 2]
                nc.sync.dma_start(s_sb[:, s0:s1], ss[:, s0:s1])
                nc.sync.dma_start(
                    w_sb[:, t0:t1].rearrange("p t c n -> p (t c n)"),
                    ws[:, t0 * 2 * P : t1 * 2 * P],
                )
                nc.sync.dma_start(
                    x_sb[:, xoff[t0] : xoff[t1]], xt[:, xoff[t0] : xoff[t1]]
                )
            nc.sync.dma_start(bias_sb[:], biasr[:])

            # pre-zero the two PSUM pool buffers: pair-fused drains read pad
            # columns no matmul ever writes; virgin PSUM could be NaN.
            for _ in range(2):
                pz = ppool.tile([P, 2, BL, 256], f32, tag="ps")
                nc.scalar.mul(
                    pz[:].rearrange("p a b c -> p (a b c)"),
                    pz[:].rearrange("p a b c -> p (a b c)"),
                    0.0,
                )


            for q in range(NT // 2):  # tile pairs
                f_bf = fpool.tile([P, 2, BL, fdq[q]], bf16, tag="f")
                ps = ppool.tile([P, 2, BL, 256], f32, tag="ps")
                for tp in range(2):
                    t = 2 * q + tp
                    for bn in tiles[t]:
                        bin0 = bn[0][5]
                        for ch in range(2):
                            for bp in range(2):
                                for ir, (br, rmin, nr, xmin, xl, coff) in enumerate(bn):
                                    o = coff - bin0
                                    base = (xoff[t]
                                            + (ch * BL + 2 * bp) * fdt[t])
                                    mov = (
                                        x_sb[:, base : base + 2 * fdt[t]]
                                        .rearrange("p (b d) -> p b d", b=2)
                                        [:, :, coff : coff + nr * xl]
                                        .rearrange(
                                            "p b (r xx) -> p b r xx", xx=xl
                                        )
                                    )
                                    nc.tensor.matmul(
                                        ps[:, tp, 2 * bp : 2 * bp + 2, o : o + nr * xl],
                                        w_sb[:, t, ch, :],
                                        mov,
                                        start=(ch == 0 and ir == 0),
                                        stop=(ch == 1 and ir == len(bn) - 1),
                                        skip_group_check=True,
                                    )
                nc.scalar.copy(
                    f_bf[:],
                    ps[:, :, :, 0 : fdq[q]],
                )
                u = upool.tile([P, 2, BL, fdq[q]], bf16, tag="u")
                nc.vector.tensor_tensor(
                    out=u[:],
                    in0=f_bf[:],
                    in1=s_sb[:, soff[q] : soff[q + 1]]
                    .rearrange("p (t d) -> p t d", t=2)
                    .unsqueeze(2)
                    .broadcast_to([P, 2, BL, fdq[q]]),
                    op=mybir.AluOpType.mult,
                )
                nc.vector.tensor_reduce(
                    out=z_sb[:, 2 * q : 2 * q + 2],
                    in_=u[:],
                    axis=mybir.AxisListType.X,
                    op=mybir.AluOpType.add,
                )

            # epilogue: z += bias (broadcast over b); out = elu(z) + 1
            zf = cpool.tile([P, NT * BL], f32)
            ze = cpool.tile([P, NT * BL], f32)
            nc.vector.tensor_tensor(
                out=zf[:].rearrange("p (t b) -> p t b", b=BL),
                in0=z_sb[:],
                in1=bias_sb[:].unsqueeze(-1).broadcast_to([P, NT, BL]),
                op=mybir.AluOpType.add,
            )
            nc.vector.tensor_scalar_min(ze[:], zf[:], 0.0)
            nc.scalar.activation(ze[:], ze[:], mybir.ActivationFunctionType.Exp)
            nc.vector.tensor_scalar_max(zf[:], zf[:], 0.0)
            nc.vector.tensor_add(zf[:], zf[:], ze[:])
            nc.sync.dma_start(out[:], zf[:])

    from concourse.library_overlay import lower_extended_insts

    lower_extended_insts(nc)
    _dedupe_ldweights(nc)
    _split_multi_waits(nc)
    nc.finalize()
    return nc


def _dedupe_ldweights(nc):
    """Drop an InstLdweights when the immediately preceding Ldweights on the
    PE loaded the exact same stationary operand and the duplicate carries no
    sync actions.  The PE keeps its foreground weights across matmuls, and
    pulled-ahead loads go to the background buffer, so reuse is safe."""
    import concourse.mybir as mybir

    def sig(ins):
        a = ins.ins[0]
        return (str(a.memref), a.offset, str(a.ap), str(a.dtype))

    for fn in nc.m.functions:
        for blk in fn.blocks:
            out = []
            last = None
            for ins in blk.instructions:
                if isinstance(ins, mybir.InstLdweights):
                    s = sig(ins)
                    si = getattr(ins, "sync_info", None)
                    clean = si is None or (not si.on_wait and not si.on_update)
                    if s == last and clean:
                        continue
                    last = s
                out.append(ins)
            blk.instructions[:] = out


def _split_multi_waits(nc):
    """The walrus build in this environment only supports ONE sync-wait slot
    per instruction.  Hoist extra waits onto NoOps inserted just before the
    offending instruction (same engine, so sequencer order enforces them)."""
    import concourse.mybir as mybir
    import bass_rust

    for fn in nc.m.functions:
        for blk in fn.blocks:
            new_insts = []
            for ins in blk.instructions:
                si = getattr(ins, "sync_info", None)
                waits = list(si.on_wait) if si is not None else []
                if len(waits) > 1:
                    for j, w in enumerate(waits[:-1]):
                        nop = mybir.InstNoOp(name=f"{ins.name}-w{j}")
                        nop.engine = ins.engine
                        nop.sync_info = bass_rust.SyncInfo(
                            on_wait=[w], on_update=[]
                        )
                        new_insts.append(nop)
                    ins.sync_info = bass_rust.SyncInfo(
                        on_wait=[waits[-1]], on_update=list(si.on_update)
                    )
                new_insts.append(ins)
            blk.instructions[:] = new_insts


def _host_prep(x, mu, W, b):
    bf16 = ml_dtypes.bfloat16

    # --- per-neuron bilinear indices / weights ---
    gx = np.clip(mu[:, 0].astype(np.float64), -1.0, 1.0)
    gy = np.clip(mu[:, 1].astype(np.float64), -1.0, 1.0)
    ix = (gx + 1.0) * (Wd * 0.5) - 0.5
    iy = (gy + 1.0) * (H * 0.5) - 0.5
    x0 = np.floor(ix)
    y0 = np.floor(iy)
    wx1 = (ix - x0).astype(np.float32)
    wy1 = (iy - y0).astype(np.float32)
    wx0 = 1.0 - wx1
    wy0 = 1.0 - wy1
    x0i = np.clip(x0.astype(np.int32), 0, Wd - 2)
    y0i = np.clip(y0.astype(np.int32), 0, H - 2)

    # sort by (block-row, x0, y0): tiles become narrow x-windows within an
    # 8-row band -> mostly one small rect each
    order = np.lexsort((y0i, x0i, y0i // 8))
    y0s, x0s = y0i[order], x0i[order]
    w4 = np.stack(
        [wx0 * wy0, wx1 * wy0, wx0 * wy1, wx1 * wy1], axis=-1
    ).astype(np.float32)[order]

    # --- per-tile rects (grouped by block-row), first-fit bins <= 256 cols ---
    tiles = []    # per tile: list of bins; bin = [(br, rmin, nr, xmin, xl, coloff)]
    fdt = []
    for t in range(NT):
        sl = slice(t * P, (t + 1) * P)
        yy, xx = y0s[sl], x0s[sl]
        rows = np.concatenate([yy, yy + 1])
        xs = np.concatenate([xx, xx])
        pts_all = list(zip(rows, xs))

        def make_rects(br, pts):
            rmin = min(p[0] for p in pts)
            rmax = max(p[0] for p in pts)
            xmin = min(p[1] for p in pts)
            xmax = max(p[1] for p in pts)
            nr = rmax - rmin + 1
            xl = xmax - xmin + 2
            xl += xl & 1  # pad to even for bf16 2x alignment
            if xl > Wd - xmin:
                xl = Wd - xmin  # halo fits (x0<=60) so only pad can overflow
            if nr * xl <= 256:
                return [(br, rmin, nr, xmin, xl)]
            # split at the largest x gap (fall back to median x)
            xsrt = sorted({p[1] for p in pts})
            gaps = [(xsrt[i + 1] - xsrt[i], xsrt[i]) for i in range(len(xsrt) - 1)]
            gmax = max(gaps)
            cut = gmax[1] if gmax[0] > 1 else xsrt[len(xsrt) // 2 - 1]
            lo = [p for p in pts if p[1] <= cut]
            hi = [p for p in pts if p[1] > cut]
            assert lo and hi, (t, br, cut)
            return make_rects(br, lo) + make_rects(br, hi)

        rects = make_rects(0, pts_all)
        # first-fit into bins of <= 256 cols
        bins = []
        for r in rects:
            sz = r[2] * r[4]
            for bn in bins:
                if bn[0] + sz <= 256:
                    bn[0] += sz
                    bn[1].append(r)
                    break
            else:
                bins.append([sz, [r]])
        # assign column offsets (contiguous across bins)
        col = 0
        obins = []
        for _, rs in bins:
            orl = []
            for (br, rmin, nr, xmin, xl) in rs:
                orl.append((br, rmin, nr, xmin, xl, col))
                col += nr * xl
            obins.append(orl)
        tiles.append(obins)
        fdt.append(col)

    # pair padding for rectangular DVE ops
    fdq = [max(fdt[2 * q], fdt[2 * q + 1]) for q in range(NT // 2)]
    fdq = [f + (f & 1) for f in fdq]
    soff = np.cumsum([0] + [2 * f for f in fdq]).tolist()
    ssz = soff[-1]

    # --- S' (bilinear weights over rect cols), pair-padded layout ---
    ss_np = np.zeros((P, ssz), dtype=np.float32)
    for t in range(NT):
        q, tp = t // 2, t % 2
        base = soff[q] + tp * fdq[q]
        sl = slice(t * P, (t + 1) * P)
        yy, xx, wg = y0s[sl], x0s[sl], w4[sl]
        allrects = [r for bn in tiles[t] for r in bn]
        for j in range(P):
            for (r, xc, wv) in (
                (yy[j], xx[j], wg[j, 0]),
                (yy[j], xx[j] + 1, wg[j, 1]),
                (yy[j] + 1, xx[j], wg[j, 2]),
                (yy[j] + 1, xx[j] + 1, wg[j, 3]),
            ):
                for (br, rmin, nr, xmin, xl, coff) in allrects:
                    if rmin <= r < rmin + nr \
                            and xmin <= xc < xmin + xl:
                        ss_np[j, base + coff + (r - rmin) * xl
                              + (xc - xmin)] += wv
                        break
                else:
                    raise AssertionError((t, j, r, xc))
    ss_np = ss_np.astype(bf16)

    # --- W stationary: [c_part, t, ch, n] ---
    Wp = W[order].astype(bf16)  # [N, C]
    ws_np = np.ascontiguousarray(
        Wp.reshape(NT, P, 2, P)        # [t, n, ch, c_part]
        .transpose(3, 0, 2, 1)         # [c_part, t, ch, n]
        .reshape(P, NT * 2 * P)
    )
    biasr_np = np.ascontiguousarray(b[order].astype(np.float32).reshape(NT, P).T)

    # --- per-core x in TILE-RECT order: each tile's rect pixels contiguous
    # [cp, ch, b, rect-cols] so pair-0's x dependency is ~0.2 MB, every x
    # DMA is contiguous, and the moving AP reads [b-pair, nr, xl] slices.
    xoff = np.cumsum([0] + [2 * BL * f for f in fdt]).tolist()
    xsz = xoff[-1]
    xb4 = x.astype(bf16).reshape(B, 2, P, H, Wd)        # [b, ch, cp, row, x]
    xts = []
    for cix in range(NCORES):
        xc = xb4[cix * BL : (cix + 1) * BL]             # [BL, 2, cp, row, x]
        xt_np = np.empty((P, xsz), dtype=bf16)
        for t in range(NT):
            ft = fdt[t]
            for bn in tiles[t]:
                for (br, rmin, nr, xmin, xl, coff) in bn:
                    patch = (
                        xc[:, :, :, rmin : rmin + nr, xmin : xmin + xl]
                        .transpose(2, 1, 0, 3, 4)       # [cp, ch, b, nr, xl]
                        .reshape(P, 2, BL, nr * xl)
                    )
                    base = xoff[t] + coff
                    for ch in range(2):
                        for bl in range(BL):
                            xt_np[:, base + (ch * BL + bl) * ft
                                  : base + (ch * BL + bl) * ft + nr * xl] = \
                                patch[:, ch, bl]
        xts.append(xt_np)

    meta = {"tiles": tiles, "fdt": fdt, "fdq": fdq, "soff": soff, "ssz": ssz,
            "xoff": xoff, "xsz": xsz}
    shared = {"ss": ss_np, "ws": ws_np, "biasr": biasr_np}
    in_maps = [{"xt": xts[cix], **shared} for cix in range(NCORES)]
    return in_maps, meta, order


def _run(prep, trace=False, **kwargs):
    global _PROGRAM
    from concourse import bass_utils

    in_maps, meta, order = prep
    if _PROGRAM is None:
        _PROGRAM = _build_program(meta)
    rr = bass_utils.run_bass_kernel_spmd(
        _PROGRAM, in_maps, core_ids=list(range(NCORES)), trace=trace, **kwargs
    )
    inv = np.empty(N, dtype=np.int64)
    inv[order] = np.arange(N)
    outs = []
    for cix in range(NCORES):
        o = np.asarray(rr.results[cix]["out"], dtype=np.float32)  # [P, NT*BL]
        o = o.reshape(P, NT, BL).transpose(2, 1, 0).reshape(BL, N)  # sorted order
        outs.append(o[:, inv])
    return np.concatenate(outs, axis=0), rr


def kernel(x, mu, W, b):
    prep = _host_prep(x, mu, W, b)
    out, _ = _run(prep)
    return out


# revision 38
# speedup vs baseline: 1.1311x; 1.1311x over previous
"""Trainium2 Bass kernel for PoissonGaussianReadout.

Computation (per reference):
  out[b, n] = elu( sum_c bilinear_sample(x[b, c], mu[n]) * W[n, c] + bias[n] ) + 1

Sharding: data-parallel over batch B=32 across 8 cores (4 images per core).
Every core processes all N=8192 neurons for its 4 images.

Device strategy per core ("rect-B": channel dot on TensorE first, then
bilinear interpolation as a small DVE weighted sum; 73us vs 477us for the
dma_gather baseline):
  - Neurons are host-sorted by (y0//8, x0, y0), packed into 64 tiles of
    128.  A tile's corner pixels then span one small rectangle (~9x12 px,
    FD~84 avg; recursively split only if over a PSUM half-bank).
  - x is host-transposed channel-major: x[c_part, c_chunk, b, pix],
    pixel-contiguous so a rect AP spans rows freely.
  - TensorE: Y[slot, b, rect-px] = sum_c W[slot, c] * x[c, b, rect-px] —
    stationary = per-tile W chunk [128c x 128n], moving = the rect pixels,
    PSUM-accumulated over the 2 c-chunks.  Per-b-pair matmuls keep each
    output inside one PSUM bank.
  - ScalarE drains PSUM f32 -> SBUF bf16 per bin (rects first-fit packed
    into <=256-col PSUM half-bank bins).
  - DVE: z[slot, b] = sum_px Y[slot, b, px] * S'[slot, px] where S' holds
    the 4 bilinear corner weights (zero elsewhere) — tensor_tensor mult
    (2x bf16) + tensor_reduce, per tile pair.
  - Epilogue: out = exp(min(z+bias,0)) + max(z+bias,0)  (== elu(z)+1).
"""

import numpy as np
import ml_dtypes

B, C, H, Wd, N = 32, 256, 64, 64, 8192
NCORES = 8
BL = B // NCORES          # 4 images per core
P = 128                   # partitions / neurons per tile
NT = N // P               # 64 neuron tiles
NPB = 4                   # x DMA chunks
NPIX = H * Wd             # 4096
PBSZ = 1024               # pixels per x DMA chunk

_PROGRAM = None


def _build_program(meta):
    """meta: dict with per-tile rect/bin structure (see _host_prep)."""
    import concourse.bass as bass
    import concourse.mybir as mybir
    import concourse.tile as tile

    bf16 = mybir.dt.bfloat16
    f32 = mybir.dt.float32

    tiles = meta["tiles"]        # per tile: list of bins; bin = list of rects
                                 # rect = (br, rmin, nr, xmin, xl, coloff)
    fdt = meta["fdt"]            # per tile FD (cols)
    fdq = meta["fdq"]            # per pair padded FD
    soff = meta["soff"]          # per pair S' offset (elements per partition)
    ssz = meta["ssz"]
    fdmax = max(fdq)

    nc = bass.Bass("TRN2")

    xt = nc.dram_tensor("xt", [P, NPB * 2 * BL * PBSZ], bf16, kind="ExternalInput")
    ws = nc.dram_tensor("ws", [P, NT * 2 * P], bf16, kind="ExternalInput")
    ss = nc.dram_tensor("ss", [P, ssz], bf16, kind="ExternalInput")
    biasr = nc.dram_tensor("biasr", [P, NT], f32, kind="ExternalInput")
    out = nc.dram_tensor("out", [P, NT * BL], f32, kind="ExternalOutput")

    TG = 8  # tiles per S/W DMA chunk
    XB = [0, 640, 1664, 2688, 3712, 4096]  # x chunk bounds: small first chunk

    with tile.TileContext(nc) as tc:
        with (
            tc.tile_pool(name="const", bufs=1) as cpool,
            tc.tile_pool(name="fpool", bufs=3) as fpool,
            tc.tile_pool(name="upool", bufs=3) as upool,
            tc.tile_pool(name="psum", bufs=2, space="PSUM") as ppool,
        ):
            x_sb = cpool.tile([P, 2, BL, NPIX], bf16)
            s_sb = cpool.tile([P, ssz], bf16)
            w_sb = cpool.tile([P, NT, 2, P], bf16)
            bias_sb = cpool.tile([P, NT], f32)
            z_sb = cpool.tile([P, NT, BL], f32)

            # DMA interleave: fine-grained chunks early so tile-0 deps land
            # fast, coarser later.  Chunks are tile ranges.
            chunks = [(0, 2), (2, 4), (4, 8)] + [
                (t, t + TG) for t in range(8, NT, TG)
            ]
            pbdone = 0
            for t0, t1 in chunks:
                s0, s1 = soff[t0 // 2], soff[t1 // 2]
                nc.sync.dma_start(s_sb[:, s0:s1], ss[:, s0:s1])
                nc.sync.dma_start(
                    w_sb[:, t0:t1].rearrange("p t c n -> p (t c n)"),
                    ws[:, t0 * 2 * P : t1 * 2 * P],
                )
                need_px = max(
                    (r[1] + r[2] + 1) * Wd
                    for t in range(t0, t1) for bn in tiles[t] for r in bn
                )
                while pbdone < len(XB) - 1 and XB[pbdone] < need_px:
                    lo, hi = XB[pbdone], XB[pbdone + 1]
                    nc.sync.dma_start(
                        x_sb[:, :, :, lo:hi],
                        xt[:].rearrange(
                            "p (c b k) -> p c b k", c=2, b=BL
                        )[:, :, :, lo:hi],
                    )
                    pbdone += 1
            while pbdone < len(XB) - 1:
                lo, hi = XB[pbdone], XB[pbdone + 1]
                nc.sync.dma_start(
                    x_sb[:, :, :, lo:hi],
                    xt[:].rearrange(
                        "p (c b k) -> p c b k", c=2, b=BL
                    )[:, :, :, lo:hi],
                )
                pbdone += 1
            nc.sync.dma_start(bias_sb[:], biasr[:])

            # pre-zero the two PSUM pool buffers: pair-fused drains read pad
            # columns no matmul ever writes; virgin PSUM could be NaN.
            for _ in range(2):
                pz = ppool.tile([P, 2, BL, 256], f32, tag="ps")
                nc.scalar.mul(
                    pz[:].rearrange("p a b c -> p (a b c)"),
                    pz[:].rearrange("p a b c -> p (a b c)"),
                    0.0,
                )

            # x view for moving operands: [P, ch, b, row, x]
            x_r = x_sb[:].rearrange("p c b (r x) -> p c b r x", x=Wd)

            for q in range(NT // 2):  # tile pairs
                f_bf = fpool.tile([P, 2, BL, fdq[q]], bf16, tag="f")
                ps = ppool.tile([P, 2, BL, 256], f32, tag="ps")
                for tp in range(2):
                    t = 2 * q + tp
                    for bn in tiles[t]:
                        bin0 = bn[0][5]
                        for ch in range(2):
                            for bp in range(2):
                                for ir, (br, rmin, nr, xmin, xl, coff) in enumerate(bn):
                                    o = coff - bin0
                                    nc.tensor.matmul(
                                        ps[:, tp, 2 * bp : 2 * bp + 2, o : o + nr * xl],
                                        w_sb[:, t, ch, :],
                                        x_r[
                                            :, ch, 2 * bp : 2 * bp + 2,
                                            rmin : rmin + nr,
                                            xmin : xmin + xl,
                                        ],
                                        start=(ch == 0 and ir == 0),
                                        stop=(ch == 1 and ir == len(bn) - 1),
                                        skip_group_check=True,
                                    )
                nc.scalar.copy(
                    f_bf[:],
                    ps[:, :, :, 0 : fdq[q]],
                )
                u = upool.tile([P, 2, BL, fdq[q]], bf16, tag="u")
                nc.vector.tensor_tensor(
                    out=u[:],
                    in0=f_bf[:],
                    in1=s_sb[:, soff[q] : soff[q + 1]]
                    .rearrange("p (t d) -> p t d", t=2)
                    .unsqueeze(2)
                    .broadcast_to([P, 2, BL, fdq[q]]),
                    op=mybir.AluOpType.mult,
                )
                nc.vector.tensor_reduce(
                    out=z_sb[:, 2 * q : 2 * q + 2],
                    in_=u[:],
                    axis=mybir.AxisListType.X,
                    op=mybir.AluOpType.add,
                )

            # epilogue: z += bias (broadcast over b); out = elu(z) + 1
            zf = cpool.tile([P, NT * BL], f32)
            ze = cpool.tile([P, NT * BL], f32)
            nc.vector.tensor_tensor(
                out=zf[:].rearrange("p (t b) -> p t b", b=BL),
                in0=z_sb[:],
                in1=bias_sb[:].unsqueeze(-1).broadcast_to([P, NT, BL]),
                op=mybir.AluOpType.add,
            )
            nc.vector.tensor_scalar_min(ze[:], zf[:], 0.0)
            nc.scalar.activation(ze[:], ze[:], mybir.ActivationFunctionType.Exp)
            nc.vector.tensor_scalar_max(zf[:], zf[:], 0.0)
            nc.vector.tensor_add(zf[:], zf[:], ze[:])
            nc.sync.dma_start(out[:], zf[:])

    from concourse.library_overlay import lower_extended_insts

    lower_extended_insts(nc)
    _dedupe_ldweights(nc)
    _split_multi_waits(nc)
    nc.finalize()
    return nc


def _dedupe_ldweights(nc):
    """Drop an InstLdweights when the immediately preceding Ldweights on the
    PE loaded the exact same stationary operand and the duplicate carries no
    sync actions.  The PE keeps its foreground weights across matmuls, and
    pulled-ahead loads go to the background buffer, so reuse is safe."""
    import concourse.mybir as mybir

    def sig(ins):
        a = ins.ins[0]
        return (str(a.memref), a.offset, str(a.ap), str(a.dtype))

    for fn in nc.m.functions:
        for blk in fn.blocks:
            out = []
            last = None
            for ins in blk.instructions:
                if isinstance(ins, mybir.InstLdweights):
                    s = sig(ins)
                    si = getattr(ins, "sync_info", None)
                    clean = si is None or (not si.on_wait and not si.on_update)
                    if s == last and clean:
                        continue
                    last = s
                out.append(ins)
            blk.instructions[:] = out


def _split_multi_waits(nc):
    """The walrus build in this environment only supports ONE sync-wait slot
    per instruction.  Hoist extra waits onto NoOps inserted just before the
    offending instruction (same engine, so sequencer order enforces them)."""
    import concourse.mybir as mybir
    import bass_rust

    for fn in nc.m.functions:
        for blk in fn.blocks:
            new_insts = []
            for ins in blk.instructions:
                si = getattr(ins, "sync_info", None)
                waits = list(si.on_wait) if si is not None else []
                if len(waits) > 1:
                    for j, w in enumerate(waits[:-1]):
                        nop = mybir.InstNoOp(name=f"{ins.name}-w{j}")
                        nop.engine = ins.engine
                        nop.sync_info = bass_rust.SyncInfo(
                            on_wait=[w], on_update=[]
                        )
                        new_insts.append(nop)
                    ins.sync_info = bass_rust.SyncInfo(
                        on_wait=[waits[-1]], on_update=list(si.on_update)
                    )
                new_insts.append(ins)
            blk.instructions[:] = new_insts


def _host_prep(x, mu, W, b):
    bf16 = ml_dtypes.bfloat16

    # --- per-neuron bilinear indices / weights ---
    gx = np.clip(mu[:, 0].astype(np.float64), -1.0, 1.0)
    gy = np.clip(mu[:, 1].astype(np.float64), -1.0, 1.0)
    ix = (gx + 1.0) * (Wd * 0.5) - 0.5
    iy = (gy + 1.0) * (H * 0.5) - 0.5
    x0 = np.floor(ix)
    y0 = np.floor(iy)
    wx1 = (ix - x0).astype(np.float32)
    wy1 = (iy - y0).astype(np.float32)
    wx0 = 1.0 - wx1
    wy0 = 1.0 - wy1
    x0i = np.clip(x0.astype(np.int32), 0, Wd - 2)
    y0i = np.clip(y0.astype(np.int32), 0, H - 2)

    # sort by (block-row, x0, y0): tiles become narrow x-windows within an
    # 8-row band -> mostly one small rect each
    order = np.lexsort((y0i, x0i, y0i // 8))
    y0s, x0s = y0i[order], x0i[order]
    w4 = np.stack(
        [wx0 * wy0, wx1 * wy0, wx0 * wy1, wx1 * wy1], axis=-1
    ).astype(np.float32)[order]

    # --- per-tile rects (grouped by block-row), first-fit bins <= 256 cols ---
    tiles = []    # per tile: list of bins; bin = [(br, rmin, nr, xmin, xl, coloff)]
    fdt = []
    for t in range(NT):
        sl = slice(t * P, (t + 1) * P)
        yy, xx = y0s[sl], x0s[sl]
        rows = np.concatenate([yy, yy + 1])
        xs = np.concatenate([xx, xx])
        pts_all = list(zip(rows, xs))

        def make_rects(br, pts):
            rmin = min(p[0] for p in pts)
            rmax = max(p[0] for p in pts)
            xmin = min(p[1] for p in pts)
            xmax = max(p[1] for p in pts)
            nr = rmax - rmin + 1
            xl = xmax - xmin + 2
            xl += xl & 1  # pad to even for bf16 2x alignment
            if xl > Wd - xmin:
                xl = Wd - xmin  # halo fits (x0<=60) so only pad can overflow
            if nr * xl <= 256:
                return [(br, rmin, nr, xmin, xl)]
            # split at the largest x gap (fall back to median x)
            xsrt = sorted({p[1] for p in pts})
            gaps = [(xsrt[i + 1] - xsrt[i], xsrt[i]) for i in range(len(xsrt) - 1)]
            gmax = max(gaps)
            cut = gmax[1] if gmax[0] > 1 else xsrt[len(xsrt) // 2 - 1]
            lo = [p for p in pts if p[1] <= cut]
            hi = [p for p in pts if p[1] > cut]
            assert lo and hi, (t, br, cut)
            return make_rects(br, lo) + make_rects(br, hi)

        rects = make_rects(0, pts_all)
        # first-fit into bins of <= 256 cols
        bins = []
        for r in rects:
            sz = r[2] * r[4]
            for bn in bins:
                if bn[0] + sz <= 256:
                    bn[0] += sz
                    bn[1].append(r)
                    break
            else:
                bins.append([sz, [r]])
        # assign column offsets (contiguous across bins)
        col = 0
        obins = []
        for _, rs in bins:
            orl = []
            for (br, rmin, nr, xmin, xl) in rs:
                orl.append((br, rmin, nr, xmin, xl, col))
                col += nr * xl
            obins.append(orl)
        tiles.append(obins)
        fdt.append(col)

    # pair padding for rectangular DVE ops
    fdq = [max(fdt[2 * q], fdt[2 * q + 1]) for q in range(NT // 2)]
    fdq = [f + (f & 1) for f in fdq]
    soff = np.cumsum([0] + [2 * f for f in fdq]).tolist()
    ssz = soff[-1]

    # --- S' (bilinear weights over rect cols), pair-padded layout ---
    ss_np = np.zeros((P, ssz), dtype=np.float32)
    for t in range(NT):
        q, tp = t // 2, t % 2
        base = soff[q] + tp * fdq[q]
        sl = slice(t * P, (t + 1) * P)
        yy, xx, wg = y0s[sl], x0s[sl], w4[sl]
        allrects = [r for bn in tiles[t] for r in bn]
        for j in range(P):
            for (r, xc, wv) in (
                (yy[j], xx[j], wg[j, 0]),
                (yy[j], xx[j] + 1, wg[j, 1]),
                (yy[j] + 1, xx[j], wg[j, 2]),
                (yy[j] + 1, xx[j] + 1, wg[j, 3]),
            ):
                for (br, rmin, nr, xmin, xl, coff) in allrects:
                    if rmin <= r < rmin + nr \
                            and xmin <= xc < xmin + xl:
                        ss_np[j, base + coff + (r - rmin) * xl
                              + (xc - xmin)] += wv
                        break
                else:
                    raise AssertionError((t, j, r, xc))
    ss_np = ss_np.astype(bf16)

    # --- W stationary: [c_part, t, ch, n] ---
    Wp = W[order].astype(bf16)  # [N, C]
    ws_np = np.ascontiguousarray(
        Wp.reshape(NT, P, 2, P)        # [t, n, ch, c_part]
        .transpose(3, 0, 2, 1)         # [c_part, t, ch, n]
        .reshape(P, NT * 2 * P)
    )
    biasr_np = np.ascontiguousarray(b[order].astype(np.float32).reshape(NT, P).T)

    # --- per-core x channel-major: [c_part, pb, ch, b, pix-in-block] ---
    xb = x.astype(bf16).reshape(B, C, NPIX)
    xts = []
    for cix in range(NCORES):
        xc = xb[cix * BL : (cix + 1) * BL]              # [BL, C, NPIX]
        xt_np = np.ascontiguousarray(
            xc.reshape(BL, 2, P, NPIX)                  # [b, ch, cp, pix]
            .transpose(2, 1, 0, 3)                      # [cp, ch, b, pix]
            .reshape(P, 2 * BL * NPIX)
        )
        xts.append(xt_np)

    meta = {"tiles": tiles, "fdt": fdt, "fdq": fdq, "soff": soff, "ssz": ssz}
    shared = {"ss": ss_np, "ws": ws_np, "biasr": biasr_np}
    in_maps = [{"xt": xts[cix], **shared} for cix in range(NCORES)]
    return in_maps, meta, order


def _run(prep, trace=False, **kwargs):
    global _PROGRAM
    from concourse import bass_utils

    in_maps, meta, order = prep
    if _PROGRAM is None:
        _PROGRAM = _build_program(meta)
    rr = bass_utils.run_bass_kernel_spmd(
        _PROGRAM, in_maps, core_ids=list(range(NCORES)), trace=trace, **kwargs
    )
    inv = np.empty(N, dtype=np.int64)
    inv[order] = np.arange(N)
    outs = []
    for cix in range(NCORES):
        o = np.asarray(rr.results[cix]["out"], dtype=np.float32)  # [P, NT*BL]
        o = o.reshape(P, NT, BL).transpose(2, 1, 0).reshape(BL, N)  # sorted order
        outs.append(o[:, inv])
    return np.concatenate(outs, axis=0), rr


def kernel(x, mu, W, b):
    prep = _host_prep(x, mu, W, b)
    out, _ = _run(prep)
    return out


# revision 39
# speedup vs baseline: 1.1431x; 1.0106x over previous
"""Trainium2 Bass kernel for PoissonGaussianReadout.

Computation (per reference):
  out[b, n] = elu( sum_c bilinear_sample(x[b, c], mu[n]) * W[n, c] + bias[n] ) + 1

Sharding: data-parallel over batch B=32 across 8 cores (4 images per core).
Every core processes all N=8192 neurons for its 4 images.

Device strategy per core ("rect-B": channel dot on TensorE first, then
bilinear interpolation as a small DVE weighted sum; 73us vs 477us for the
dma_gather baseline):
  - Neurons are host-sorted by (y0//8, x0, y0), packed into 64 tiles of
    128.  A tile's corner pixels then span one small rectangle (~9x12 px,
    FD~84 avg; recursively split only if over a PSUM half-bank).
  - x is host-transposed channel-major: x[c_part, c_chunk, b, pix],
    pixel-contiguous so a rect AP spans rows freely.
  - TensorE: Y[slot, b, rect-px] = sum_c W[slot, c] * x[c, b, rect-px] —
    stationary = per-tile W chunk [128c x 128n], moving = the rect pixels,
    PSUM-accumulated over the 2 c-chunks.  Per-b-pair matmuls keep each
    output inside one PSUM bank.
  - ScalarE drains PSUM f32 -> SBUF bf16 per bin (rects first-fit packed
    into <=256-col PSUM half-bank bins).
  - DVE: z[slot, b] = sum_px Y[slot, b, px] * S'[slot, px] where S' holds
    the 4 bilinear corner weights (zero elsewhere) — tensor_tensor mult
    (2x bf16) + tensor_reduce, per tile pair.
  - Epilogue: out = exp(min(z+bias,0)) + max(z+bias,0)  (== elu(z)+1).
"""

import numpy as np
import ml_dtypes

B, C, H, Wd, N = 32, 256, 64, 64, 8192
NCORES = 8
BL = B // NCORES          # 4 images per core
P = 128                   # partitions / neurons per tile
NT = N // P               # 64 neuron tiles
NPB = 4                   # x DMA chunks
NPIX = H * Wd             # 4096
PBSZ = 1024               # pixels per x DMA chunk

_PROGRAM = None


def _build_program(meta):
    """meta: dict with per-tile rect/bin structure (see _host_prep)."""
    import concourse.bass as bass
    import concourse.mybir as mybir
    import concourse.tile as tile

    bf16 = mybir.dt.bfloat16
    f32 = mybir.dt.float32

    tiles = meta["tiles"]        # per tile: list of bins; bin = list of rects
                                 # rect = (br, rmin, nr, xmin, xl, coloff)
    fdt = meta["fdt"]            # per tile FD (cols)
    fdq = meta["fdq"]            # per pair padded FD
    soff = meta["soff"]          # per pair S' offset (elements per partition)
    ssz = meta["ssz"]
    fdmax = max(fdq)

    nc = bass.Bass("TRN2")

    xt = nc.dram_tensor("xt", [P, NPB * 2 * BL * PBSZ], bf16, kind="ExternalInput")
    ws = nc.dram_tensor("ws", [P, NT * 2 * P], bf16, kind="ExternalInput")
    ss = nc.dram_tensor("ss", [P, ssz], bf16, kind="ExternalInput")
    biasr = nc.dram_tensor("biasr", [P, NT], f32, kind="ExternalInput")
    out = nc.dram_tensor("out", [P, NT * BL], f32, kind="ExternalOutput")

    TG = 8  # tiles per S/W DMA chunk
    XB = [0, 640, 1664, 2688, 3712, 4096]  # x chunk bounds: small first chunk

    with tile.TileContext(nc) as tc:
        with (
            tc.tile_pool(name="const", bufs=1) as cpool,
            tc.tile_pool(name="fpool", bufs=4) as fpool,
            tc.tile_pool(name="upool", bufs=4) as upool,
            tc.tile_pool(name="psum", bufs=2, space="PSUM") as ppool,
        ):
            x_sb = cpool.tile([P, 2, BL, NPIX], bf16)
            s_sb = cpool.tile([P, ssz], bf16)
            w_sb = cpool.tile([P, NT, 2, P], bf16)
            bias_sb = cpool.tile([P, NT], f32)
            z_sb = cpool.tile([P, NT, BL], f32)

            # DMA interleave: fine-grained chunks early so tile-0 deps land
            # fast, coarser later.  Chunks are tile ranges.
            chunks = [(0, 2), (2, 4), (4, 8)] + [
                (t, t + TG) for t in range(8, NT, TG)
            ]
            pbdone = 0
            for t0, t1 in chunks:
                s0, s1 = soff[t0 // 2], soff[t1 // 2]
                nc.sync.dma_start(s_sb[:, s0:s1], ss[:, s0:s1])
                nc.sync.dma_start(
                    w_sb[:, t0:t1].rearrange("p t c n -> p (t c n)"),
                    ws[:, t0 * 2 * P : t1 * 2 * P],
                )
                need_px = max(
                    (r[1] + r[2] + 1) * Wd
                    for t in range(t0, t1) for bn in tiles[t] for r in bn
                )
                while pbdone < len(XB) - 1 and XB[pbdone] < need_px:
                    lo, hi = XB[pbdone], XB[pbdone + 1]
                    nc.sync.dma_start(
                        x_sb[:, :, :, lo:hi],
                        xt[:].rearrange(
                            "p (c b k) -> p c b k", c=2, b=BL
                        )[:, :, :, lo:hi],
                    )
                    pbdone += 1
            while pbdone < len(XB) - 1:
                lo, hi = XB[pbdone], XB[pbdone + 1]
                nc.sync.dma_start(
                    x_sb[:, :, :, lo:hi],
                    xt[:].rearrange(
                        "p (c b k) -> p c b k", c=2, b=BL
                    )[:, :, :, lo:hi],
                )
                pbdone += 1
            nc.sync.dma_start(bias_sb[:], biasr[:])

            # pre-zero the two PSUM pool buffers: pair-fused drains read pad
            # columns no matmul ever writes; virgin PSUM could be NaN.
            for _ in range(2):
                pz = ppool.tile([P, 2, BL, 256], f32, tag="ps")
                nc.scalar.mul(
                    pz[:].rearrange("p a b c -> p (a b c)"),
                    pz[:].rearrange("p a b c -> p (a b c)"),
                    0.0,
                )

            # x view for moving operands: [P, ch, b, row, x]
            x_r = x_sb[:].rearrange("p c b (r x) -> p c b r x", x=Wd)

            for q in range(NT // 2):  # tile pairs
                f_bf = fpool.tile([P, 2, BL, fdq[q]], bf16, tag="f")
                ps = ppool.tile([P, 2, BL, 256], f32, tag="ps")
                for tp in range(2):
                    t = 2 * q + tp
                    for bn in tiles[t]:
                        bin0 = bn[0][5]
                        for ch in range(2):
                            for bp in range(2):
                                for ir, (br, rmin, nr, xmin, xl, coff) in enumerate(bn):
                                    o = coff - bin0
                                    nc.tensor.matmul(
                                        ps[:, tp, 2 * bp : 2 * bp + 2, o : o + nr * xl],
                                        w_sb[:, t, ch, :],
                                        x_r[
                                            :, ch, 2 * bp : 2 * bp + 2,
                                            rmin : rmin + nr,
                                            xmin : xmin + xl,
                                        ],
                                        start=(ch == 0 and ir == 0),
                                        stop=(ch == 1 and ir == len(bn) - 1),
                                        skip_group_check=True,
                                    )
                nc.scalar.copy(
                    f_bf[:],
                    ps[:, :, :, 0 : fdq[q]],
                )
                u = upool.tile([P, 2, BL, fdq[q]], bf16, tag="u")
                nc.vector.tensor_tensor(
                    out=u[:],
                    in0=f_bf[:],
                    in1=s_sb[:, soff[q] : soff[q + 1]]
                    .rearrange("p (t d) -> p t d", t=2)
                    .unsqueeze(2)
                    .broadcast_to([P, 2, BL, fdq[q]]),
                    op=mybir.AluOpType.mult,
                )
                nc.vector.tensor_reduce(
                    out=z_sb[:, 2 * q : 2 * q + 2],
                    in_=u[:],
                    axis=mybir.AxisListType.X,
                    op=mybir.AluOpType.add,
                )

            # epilogue: z += bias (broadcast over b); out = elu(z) + 1
            zf = cpool.tile([P, NT * BL], f32)
            ze = cpool.tile([P, NT * BL], f32)
            nc.vector.tensor_tensor(
                out=zf[:].rearrange("p (t b) -> p t b", b=BL),
                in0=z_sb[:],
                in1=bias_sb[:].unsqueeze(-1).broadcast_to([P, NT, BL]),
                op=mybir.AluOpType.add,
            )
            nc.vector.tensor_scalar_min(ze[:], zf[:], 0.0)
            nc.scalar.activation(ze[:], ze[:], mybir.ActivationFunctionType.Exp)
            nc.vector.tensor_scalar_max(zf[:], zf[:], 0.0)
            nc.vector.tensor_add(zf[:], zf[:], ze[:])
            nc.sync.dma_start(out[:], zf[:])

    from concourse.library_overlay import lower_extended_insts

    lower_extended_insts(nc)
    _dedupe_ldweights(nc)
    _split_multi_waits(nc)
    nc.finalize()
    return nc


def _dedupe_ldweights(nc):
    """Drop an InstLdweights when the immediately preceding Ldweights on the
    PE loaded the exact same stationary operand and the duplicate carries no
    sync actions.  The PE keeps its foreground weights across matmuls, and
    pulled-ahead loads go to the background buffer, so reuse is safe."""
    import concourse.mybir as mybir

    def sig(ins):
        a = ins.ins[0]
        return (str(a.memref), a.offset, str(a.ap), str(a.dtype))

    for fn in nc.m.functions:
        for blk in fn.blocks:
            out = []
            last = None
            for ins in blk.instructions:
                if isinstance(ins, mybir.InstLdweights):
                    s = sig(ins)
                    si = getattr(ins, "sync_info", None)
                    clean = si is None or (not si.on_wait and not si.on_update)
                    if s == last and clean:
                        continue
                    last = s
                out.append(ins)
            blk.instructions[:] = out


def _split_multi_waits(nc):
    """The walrus build in this environment only supports ONE sync-wait slot
    per instruction.  Hoist extra waits onto NoOps inserted just before the
    offending instruction (same engine, so sequencer order enforces them)."""
    import concourse.mybir as mybir
    import bass_rust

    for fn in nc.m.functions:
        for blk in fn.blocks:
            new_insts = []
            for ins in blk.instructions:
                si = getattr(ins, "sync_info", None)
                waits = list(si.on_wait) if si is not None else []
                if len(waits) > 1:
                    for j, w in enumerate(waits[:-1]):
                        nop = mybir.InstNoOp(name=f"{ins.name}-w{j}")
                        nop.engine = ins.engine
                        nop.sync_info = bass_rust.SyncInfo(
                            on_wait=[w], on_update=[]
                        )
                        new_insts.append(nop)
                    ins.sync_info = bass_rust.SyncInfo(
                        on_wait=[waits[-1]], on_update=list(si.on_update)
                    )
                new_insts.append(ins)
            blk.instructions[:] = new_insts


def _host_prep(x, mu, W, b):
    bf16 = ml_dtypes.bfloat16

    # --- per-neuron bilinear indices / weights ---
    gx = np.clip(mu[:, 0].astype(np.float64), -1.0, 1.0)
    gy = np.clip(mu[:, 1].astype(np.float64), -1.0, 1.0)
    ix = (gx + 1.0) * (Wd * 0.5) - 0.5
    iy = (gy + 1.0) * (H * 0.5) - 0.5
    x0 = np.floor(ix)
    y0 = np.floor(iy)
    wx1 = (ix - x0).astype(np.float32)
    wy1 = (iy - y0).astype(np.float32)
    wx0 = 1.0 - wx1
    wy0 = 1.0 - wy1
    x0i = np.clip(x0.astype(np.int32), 0, Wd - 2)
    y0i = np.clip(y0.astype(np.int32), 0, H - 2)

    # sort by (block-row, x0, y0): tiles become narrow x-windows within an
    # 8-row band -> mostly one small rect each
    order = np.lexsort((y0i, x0i, y0i // 8))
    y0s, x0s = y0i[order], x0i[order]
    w4 = np.stack(
        [wx0 * wy0, wx1 * wy0, wx0 * wy1, wx1 * wy1], axis=-1
    ).astype(np.float32)[order]

    # --- per-tile rects (grouped by block-row), first-fit bins <= 256 cols ---
    tiles = []    # per tile: list of bins; bin = [(br, rmin, nr, xmin, xl, coloff)]
    fdt = []
    for t in range(NT):
        sl = slice(t * P, (t + 1) * P)
        yy, xx = y0s[sl], x0s[sl]
        rows = np.concatenate([yy, yy + 1])
        xs = np.concatenate([xx, xx])
        pts_all = list(zip(rows, xs))

        def make_rects(br, pts):
            rmin = min(p[0] for p in pts)
            rmax = max(p[0] for p in pts)
            xmin = min(p[1] for p in pts)
            xmax = max(p[1] for p in pts)
            nr = rmax - rmin + 1
            xl = xmax - xmin + 2
            xl += xl & 1  # pad to even for bf16 2x alignment
            if xl > Wd - xmin:
                xl = Wd - xmin  # halo fits (x0<=60) so only pad can overflow
            if nr * xl <= 256:
                return [(br, rmin, nr, xmin, xl)]
            # split at the largest x gap (fall back to median x)
            xsrt = sorted({p[1] for p in pts})
            gaps = [(xsrt[i + 1] - xsrt[i], xsrt[i]) for i in range(len(xsrt) - 1)]
            gmax = max(gaps)
            cut = gmax[1] if gmax[0] > 1 else xsrt[len(xsrt) // 2 - 1]
            lo = [p for p in pts if p[1] <= cut]
            hi = [p for p in pts if p[1] > cut]
            assert lo and hi, (t, br, cut)
            return make_rects(br, lo) + make_rects(br, hi)

        rects = make_rects(0, pts_all)
        # first-fit into bins of <= 256 cols
        bins = []
        for r in rects:
            sz = r[2] * r[4]
            for bn in bins:
                if bn[0] + sz <= 256:
                    bn[0] += sz
                    bn[1].append(r)
                    break
            else:
                bins.append([sz, [r]])
        # assign column offsets (contiguous across bins)
        col = 0
        obins = []
        for _, rs in bins:
            orl = []
            for (br, rmin, nr, xmin, xl) in rs:
                orl.append((br, rmin, nr, xmin, xl, col))
                col += nr * xl
            obins.append(orl)
        tiles.append(obins)
        fdt.append(col)

    # pair padding for rectangular DVE ops
    fdq = [max(fdt[2 * q], fdt[2 * q + 1]) for q in range(NT // 2)]
    fdq = [f + (f & 1) for f in fdq]
    soff = np.cumsum([0] + [2 * f for f in fdq]).tolist()
    ssz = soff[-1]

    # --- S' (bilinear weights over rect cols), pair-padded layout ---
    ss_np = np.zeros((P, ssz), dtype=np.float32)
    for t in range(NT):
        q, tp = t // 2, t % 2
        base = soff[q] + tp * fdq[q]
        sl = slice(t * P, (t + 1) * P)
        yy, xx, wg = y0s[sl], x0s[sl], w4[sl]
        allrects = [r for bn in tiles[t] for r in bn]
        for j in range(P):
            for (r, xc, wv) in (
                (yy[j], xx[j], wg[j, 0]),
                (yy[j], xx[j] + 1, wg[j, 1]),
                (yy[j] + 1, xx[j], wg[j, 2]),
                (yy[j] + 1, xx[j] + 1, wg[j, 3]),
            ):
                for (br, rmin, nr, xmin, xl, coff) in allrects:
                    if rmin <= r < rmin + nr \
                            and xmin <= xc < xmin + xl:
                        ss_np[j, base + coff + (r - rmin) * xl
                              + (xc - xmin)] += wv
                        break
                else:
                    raise AssertionError((t, j, r, xc))
    ss_np = ss_np.astype(bf16)

    # --- W stationary: [c_part, t, ch, n] ---
    Wp = W[order].astype(bf16)  # [N, C]
    ws_np = np.ascontiguousarray(
        Wp.reshape(NT, P, 2, P)        # [t, n, ch, c_part]
        .transpose(3, 0, 2, 1)         # [c_part, t, ch, n]
        .reshape(P, NT * 2 * P)
    )
    biasr_np = np.ascontiguousarray(b[order].astype(np.float32).reshape(NT, P).T)

    # --- per-core x channel-major: [c_part, pb, ch, b, pix-in-block] ---
    xb = x.astype(bf16).reshape(B, C, NPIX)
    xts = []
    for cix in range(NCORES):
        xc = xb[cix * BL : (cix + 1) * BL]              # [BL, C, NPIX]
        xt_np = np.ascontiguousarray(
            xc.reshape(BL, 2, P, NPIX)                  # [b, ch, cp, pix]
            .transpose(2, 1, 0, 3)                      # [cp, ch, b, pix]
            .reshape(P, 2 * BL * NPIX)
        )
        xts.append(xt_np)

    meta = {"tiles": tiles, "fdt": fdt, "fdq": fdq, "soff": soff, "ssz": ssz}
    shared = {"ss": ss_np, "ws": ws_np, "biasr": biasr_np}
    in_maps = [{"xt": xts[cix], **shared} for cix in range(NCORES)]
    return in_maps, meta, order


def _run(prep, trace=False, **kwargs):
    global _PROGRAM
    from concourse import bass_utils

    in_maps, meta, order = prep
    if _PROGRAM is None:
        _PROGRAM = _build_program(meta)
    rr = bass_utils.run_bass_kernel_spmd(
        _PROGRAM, in_maps, core_ids=list(range(NCORES)), trace=trace, **kwargs
    )
    inv = np.empty(N, dtype=np.int64)
    inv[order] = np.arange(N)
    outs = []
    for cix in range(NCORES):
        o = np.asarray(rr.results[cix]["out"], dtype=np.float32)  # [P, NT*BL]
        o = o.reshape(P, NT, BL).transpose(2, 1, 0).reshape(BL, N)  # sorted order
        outs.append(o[:, inv])
    return np.concatenate(outs, axis=0), rr


def kernel(x, mu, W, b):
    prep = _host_prep(x, mu, W, b)
    out, _ = _run(prep)
    return out
